# revision 46
# baseline (speedup 1.0000x reference)
"""Trainium2 Bass kernel for a dense cross-task transformer block.

Math notes
----------
The reference "attention" has sequence length 1 on the key axis, so
softmax(scores) == 1.0 exactly and the whole q/k/score path is dead:

    mha_len1(q_in, kv_in, ...) == (kv_in @ wv.T + bv) @ wo.T + bo

which folds (on host) into a single matmul with W = wo @ wv and
b = wo @ bv + bo.  The block is then:

    verb1 = LN(verb + noun @ W1.T + c1)          (ln_v)
    verb2 = verb1 + FFN_v(verb1)
    noun1 = LN(noun + verb2 @ W2.T + c2)         (ln_n)
    noun2 = noun1 + FFN_n(noun1)
    return verb2, noun2

Device strategy
---------------
Pure data parallel over 8 cores (batch 16384 -> 2048 cols/core), weights
replicated, everything feature-major ([E, batch]) so matmuls contract
along the SBUF partition dim.

v2 design (vs the f32r baseline):
 - all matmul operands in bf16 (fp32 PSUM accumulation) -> FastWeightLoad
   kicks in and LDWEIGHTS overlaps fully; inputs/weights uploaded bf16.
 - phases stream per 512-column chunk; verb2 stays resident in SBUF for
   phase C (no DRAM round trip).
 - FFN loops are chunk-outer so FFN chunk c starts as soon as the LN of
   chunk c lands; w1 resident per phase, w2 streamed per chunk-halfblock.
 - single 8-bank PSUM pool choreography: mains/ffn1 rotate banks 0-3,
   attn stats use 4-5, LN broadcasts 6-7, ffn2 half-blocks use 0-3/4-7.
 - LayerNorm reduces across partitions via ones-vector matmuls; -mean and
   1/std broadcast back with K=1 matmuls; scale/shift fused in one
   tensor_scalar op.
"""

import os
import numpy as np
import ml_dtypes
from contextlib import ExitStack

import concourse.bass as bass
import concourse.bacc as bacc_mod
import concourse.mybir as mybir
import concourse.tile as tile
from concourse.bass_utils import run_bass_kernel_spmd

E = 1024          # embed dim
H2 = 2048         # FFN hidden dim
B_TOTAL = 16384
NCORES = 8
B = B_TOTAL // NCORES   # 2048 cols per core
P = 128
EPS = 1e-5
CHUNK = 512
NCH = B // CHUNK  # 4
KT = E // P       # 8
MT = E // P       # 8
HT = H2 // P      # 16

F32 = mybir.dt.float32
F32R = mybir.dt.float32r
BF16 = mybir.dt.bfloat16
AF = mybir.ActivationFunctionType
OP = mybir.AluOpType


def _build_program():
    nc = bacc_mod.Bacc("TRN2", target_bir_lowering=False)

    # vTb = verb + attn1_bias, nTb = noun + attn2_bias (bias host-folded
    # into the residual); nT = raw noun features (phase-A matmul input)
    vTb = nc.declare_dram_parameter("vTb", [E, B], BF16, isOutput=False)
    nTb = nc.declare_dram_parameter("nTb", [E, B], BF16, isOutput=False)
    nT = nc.declare_dram_parameter("nT", [E, B], BF16, isOutput=False)
    wat1 = nc.declare_dram_parameter("wat1", [E, E], BF16, isOutput=False)  # (wo@wv).T
    wat2 = nc.declare_dram_parameter("wat2", [E, E], BF16, isOutput=False)
    lnvg = nc.declare_dram_parameter("lnvg", [E], F32, isOutput=False)
    lnng = nc.declare_dram_parameter("lnng", [E], F32, isOutput=False)
    w1v = nc.declare_dram_parameter("w1v", [E, H2], BF16, isOutput=False)   # fv_w1.T
    b1v = nc.declare_dram_parameter("b1v", [H2], F32, isOutput=False)
    w2v = nc.declare_dram_parameter("w2v", [H2, E], BF16, isOutput=False)   # fv_w2.T
    b2v = nc.declare_dram_parameter("b2v", [E], F32, isOutput=False)
    w1n = nc.declare_dram_parameter("w1n", [E, H2], BF16, isOutput=False)
    b1n = nc.declare_dram_parameter("b1n", [H2], F32, isOutput=False)
    w2n = nc.declare_dram_parameter("w2n", [H2, E], BF16, isOutput=False)
    b2n = nc.declare_dram_parameter("b2n", [E], F32, isOutput=False)
    ones_b_d = nc.declare_dram_parameter("ones_b_d", [P, P], BF16, isOutput=False)
    verb_out = nc.declare_dram_parameter("verb_out", [E, B], F32, isOutput=True)
    noun_out = nc.declare_dram_parameter("noun_out", [E, B], F32, isOutput=True)


    with tile.TileContext(nc) as tc, ExitStack() as ctx:
        dramp = ctx.enter_context(tc.tile_pool(name="dramp", bufs=1,
                                               space="DRAM"))
        const = ctx.enter_context(tc.tile_pool(name="const", bufs=1))
        kxp = ctx.enter_context(tc.tile_pool(name="kxp", bufs=3))
        rp = ctx.enter_context(tc.tile_pool(name="rp", bufs=3))
        v1p = ctx.enter_context(tc.tile_pool(name="v1p", bufs=1))
        v2sp = ctx.enter_context(tc.tile_pool(name="v2sp", bufs=4))
        hp = ctx.enter_context(tc.tile_pool(name="hp", bufs=2))
        wap = ctx.enter_context(tc.tile_pool(name="wap", bufs=1))
        w1p = ctx.enter_context(tc.tile_pool(name="w1p", bufs=1))
        w2sp = ctx.enter_context(tc.tile_pool(name="w2sp", bufs=3))
        stp = ctx.enter_context(tc.tile_pool(name="stp", bufs=4))
        sqp = ctx.enter_context(tc.tile_pool(name="sqp", bufs=1))
        smp = ctx.enter_context(tc.tile_pool(name="smp", bufs=1))
        psp = ctx.enter_context(tc.tile_pool(name="psp", bufs=1, space="PSUM"))

        def psum(i):
            return psp.tile([P, CHUNK], F32, tag=f"b{i}", name=f"b{i}")

        def psum_stat(i):
            return psp.tile([1, CHUNK], F32, tag=f"b{i}", name=f"b{i}")

        # full [128,128] ones as the stats lhsT: the column-sum matmul then
        # lands REPLICATED across all 128 partitions -> broadcast for free
        ones_bf = const.tile([P, P], BF16, tag="ones_bf", name="ones_bf")
        nc.sync.dma_start(out=ones_bf[:], in_=ones_b_d[:, :])
        eps_t = const.tile([P, 1], F32, tag="eps", name="eps")
        nc.vector.memset(eps_t[:], EPS)

        def load_pvec(dram_ap, ntiles, tag):
            t = const.tile([P, ntiles], F32, tag=tag, name=tag)
            nc.sync.dma_start(out=t[:], in_=dram_ap.rearrange("(t p) -> p t", p=P))
            return t

        lnvg_pb = load_pvec(lnvg[:], MT, "lnvg")
        lnng_pb = load_pvec(lnng[:], MT, "lnng")
        b1v_pb = load_pvec(b1v[:], HT, "b1v")
        b2v_pb = load_pvec(b2v[:], MT, "b2v")
        b1n_pb = load_pvec(b1n[:], HT, "b1n")
        b2n_pb = load_pvec(b2n[:], MT, "b2n")

        def load_attn_w(w_dram):
            tiles = []
            for k in range(KT):
                t = wap.tile([P, E], BF16, tag=f"aw{k}", name=f"aw{k}")
                nc.sync.dma_start(out=t[:], in_=w_dram[k * P:(k + 1) * P, :])
                tiles.append(t)
            return tiles

        def load_w1(w_dram):
            tiles = []
            for k in range(KT):
                t = w1p.tile([P, H2], BF16, tag=f"w1_{k}", name=f"w1_{k}")
                nc.sync.dma_start(out=t[:], in_=w_dram[k * P:(k + 1) * P, :])
                tiles.append(t)
            return tiles

        def attn_ln(wt, kx_dram, kx_tiles, res_dram, out_tiles):
            """out[m][:,cs] = (x - mean)/std with x = res + W@kx, per
            512-col chunk.  LN gain/shift are host-folded into the
            downstream FFN weights and residual path.

            The stats matmuls use a full ones matrix as lhsT, so the
            column sums arrive replicated across all 128 partitions — the
            whole mean/var/1-over-std chain runs on replicated [128,512]
            tiles and no partition broadcast is ever needed.  Chunk c's
            apply is emitted after chunk c+1's matmuls (software pipeline).
            """
            def ln_apply(c, nmB, rsB):
                cs = slice(c * CHUNK, (c + 1) * CHUNK)
                for m in range(MT):
                    xt = out_tiles[m][:, cs]
                    nc.vector.tensor_add(xt, xt, nmB[:])
                    nc.vector.tensor_mul(xt, xt, rsB[:])

            pending = None
            for c in range(NCH):
                cs = slice(c * CHUNK, (c + 1) * CHUNK)
                if kx_tiles is None:
                    kx = []
                    for k in range(KT):
                        t = kxp.tile([P, CHUNK], BF16, tag=f"kx{k}", name=f"kx{k}")
                        nc.sync.dma_start(out=t[:], in_=kx_dram[k * P:(k + 1) * P, cs])
                        kx.append(t[:])
                else:
                    kx = [kx_tiles[k][:, cs] for k in range(KT)]
                rt = []
                for m in range(MT):
                    t = rp.tile([P, CHUNK], BF16, tag=f"r{m}", name=f"r{m}")
                    nc.sync.dma_start(out=t[:], in_=res_dram[m * P:(m + 1) * P, cs])
                    rt.append(t)
                # stats matmuls run one m-iteration behind the mains so the
                # PE never waits on the per-m PSUM drain chain mid-stream
                stats_x = psum(4)
                stats_q = psum(5)
                sqs = []

                def stats(m):
                    nc.tensor.matmul(stats_x[:], lhsT=ones_bf[:],
                                     rhs=out_tiles[m][:, cs],
                                     start=(m == 0), stop=(m == MT - 1))
                    nc.tensor.matmul(stats_q[:], lhsT=ones_bf[:],
                                     rhs=sqs[m][:],
                                     start=(m == 0), stop=(m == MT - 1))

                for m in range(MT):
                    ps = psum((0, 1, 2, 3, 6, 7)[m % 6])
                    for k in range(KT):
                        nc.tensor.matmul(
                            ps[:], lhsT=wt[k][:, m * P:(m + 1) * P],
                            rhs=kx[k],
                            start=(k == 0), stop=(k == KT - 1))
                    xt = out_tiles[m][:, cs]
                    nc.vector.tensor_add(xt, ps[:], rt[m][:])
                    sqm = sqp.tile([P, CHUNK], BF16, tag=f"sq{m % 2}",
                                   name=f"sq{m % 2}")
                    nc.scalar.activation(sqm[:], xt, AF.Square)
                    sqs.append(sqm)
                    if m > 0:
                        stats(m - 1)
                stats(MT - 1)
                # replicated [128,512] stats -> -mean (bf16), 1/std (f32)
                nmB = smp.tile([P, CHUNK], BF16, tag=f"nb{c % 2}",
                               name=f"nb{c % 2}")
                with nc.allow_low_precision(reason="mean in bf16 is plenty"):
                    nc.vector.tensor_scalar(
                        nmB[:], stats_x[:], -1.0 / E, None, OP.mult)
                m2 = smp.tile([P, CHUNK], F32, tag="m2", name="m2")
                nc.scalar.activation(m2[:], stats_x[:], AF.Square, scale=1.0 / E)
                t1 = smp.tile([P, CHUNK], F32, tag="t1", name="t1")
                nc.vector.tensor_scalar(t1[:], stats_q[:], 1.0 / E, None, OP.mult)
                nc.vector.tensor_sub(t1[:], t1[:], m2[:])           # var
                u = smp.tile([P, CHUNK], F32, tag="m2", name="m2")
                nc.scalar.activation(u[:], t1[:], AF.Ln, bias=eps_t[:])
                rsB = smp.tile([P, CHUNK], F32, tag=f"rb{c % 2}",
                               name=f"rb{c % 2}")
                nc.scalar.activation(rsB[:], u[:], AF.Exp, scale=-0.5)
                if pending is not None:
                    ln_apply(*pending)
                pending = (c, nmB, rsB)
            ln_apply(*pending)

        def ffn(in_tiles, w1t, b1_pb, w2_dram, g_pb, b2_pb, out_dram, out_bf):
            """out = (in*g + b2') + W2.T@gelu(W1'.T@in + b1') per 512-col
            chunk.  in_tiles hold the unscaled LN output; the LN gain is
            host-folded into W1 and applied to the residual here (g_pb),
            with b2' = ln_bias + ffn_b2."""
            for c in range(NCH):
                cs = slice(c * CHUNK, (c + 1) * CHUNK)
                hts = []
                for hm in range(HT):
                    ps = psum(hm % 4)
                    for k in range(KT):
                        nc.tensor.matmul(
                            ps[:], lhsT=w1t[k][:, hm * P:(hm + 1) * P],
                            rhs=in_tiles[k][:, cs],
                            start=(k == 0), stop=(k == KT - 1))
                    ht = hp.tile([P, CHUNK], BF16, tag=f"h{hm}", name=f"h{hm}")
                    nc.scalar.activation(ht[:], ps[:], AF.Gelu,
                                         bias=b1_pb[:, hm:hm + 1])
                    hts.append(ht)
                for blk in range(2):
                    ms = range(blk * 4, blk * 4 + 4)
                    pss = [psum(blk * 4 + mi) for mi in range(4)]
                    for k in range(HT):
                        w2t = w2sp.tile([P, CHUNK], BF16, tag="w2s", name="w2s")
                        nc.sync.dma_start(
                            out=w2t[:],
                            in_=w2_dram[k * P:(k + 1) * P,
                                        blk * CHUNK:(blk + 1) * CHUNK])
                        for mi, m in enumerate(ms):
                            nc.tensor.matmul(
                                pss[mi][:], lhsT=w2t[:, mi * P:(mi + 1) * P],
                                rhs=hts[k][:],
                                start=(k == 0), stop=(k == HT - 1))
                    for mi, m in enumerate(ms):
                        st = stp.tile([P, CHUNK], F32, tag="st", name="st")
                        nc.vector.tensor_scalar(
                            st[:], in_tiles[m][:, cs], g_pb[:, m:m + 1],
                            b2_pb[:, m:m + 1], OP.mult, OP.add)
                        nc.vector.tensor_add(st[:], st[:], pss[mi][:])
                        nc.sync.dma_start(out=out_dram[m * P:(m + 1) * P, cs],
                                          in_=st[:])
                        if out_bf is not None:
                            vb = v2sp.tile([P, CHUNK], BF16, tag="v2s",
                                           name="v2s")
                            nc.scalar.activation(vb[:], st[:], AF.Copy)
                            nc.sync.dma_start(
                                out=out_bf[m * P:(m + 1) * P, cs], in_=vb[:])

        _REP = int(os.environ.get("BENCH_REPEAT", "1"))
        v2d_t = dramp.tile([E, B], BF16, tag="v2d", name="v2d")
        v2d = v2d_t[:, :]
        for _rep in range(_REP):
            # A: verb attends to noun, LN -> verb1 (SBUF resident)
            wA = load_attn_w(wat1)
            v1 = [v1p.tile([P, B], BF16, tag=f"v1_{m}", name=f"v1_{m}")
                  for m in range(MT)]
            attn_ln(wA, nT, None, vTb, v1)
            w1tv = load_w1(w1v)
            # B: verb FFN -> verb_out (DRAM, f32) + v2d (DRAM bf16)
            ffn(v1, w1tv, b1v_pb, w2v, lnvg_pb, b2v_pb, verb_out, v2d)
            # C: noun attends to verb2 (read back), LN -> noun1 (v1 slots)
            wC = load_attn_w(wat2)
            w1tn = load_w1(w1n)
            n1 = [v1p.tile([P, B], BF16, tag=f"v1_{m}", name=f"v1_{m}")
                  for m in range(MT)]
            attn_ln(wC, v2d, None, nTb, n1)
            # D: noun FFN -> noun_out
            ffn(n1, w1tn, b1n_pb, w2n, lnng_pb, b2n_pb, noun_out, None)

    nc.finalize()
    return nc


_prog_cache = {}


def _get_program():
    if "nc" not in _prog_cache:
        _prog_cache["nc"] = _build_program()
    return _prog_cache["nc"]


def _prepare_maps(inputs):
    f32 = np.float32
    bf16 = ml_dtypes.bfloat16
    g = {k: np.asarray(v, f32) for k, v in inputs.items()}

    def fold(p):
        w = g[f"{p}_wo"] @ g[f"{p}_wv"]
        b = g[f"{p}_wo"] @ g[f"{p}_bv"] + g[f"{p}_bo"]
        return np.ascontiguousarray(w.T).astype(bf16), np.ascontiguousarray(b)

    wat1, bat1 = fold("v2n")
    wat2, bat2 = fold("n2v")
    # LN gain/shift fold: FFN runs on the unscaled normalized x, with
    #   W1' = W1 * g (per input feature), b1' = b1 + W1 @ beta,
    #   residual applied as in*g + (beta + b2) on device.
    w1v = (g["fv_w1"] * g["ln_v_g"][None, :]).T
    b1v = g["fv_b1"] + g["fv_w1"] @ g["ln_v_b"]
    b2v = g["fv_b2"] + g["ln_v_b"]
    w1n = (g["fn_w1"] * g["ln_n_g"][None, :]).T
    b1n = g["fn_b1"] + g["fn_w1"] @ g["ln_n_b"]
    b2n = g["fn_b2"] + g["ln_n_b"]
    common = {
        "wat1": wat1, "wat2": wat2,
        "lnvg": g["ln_v_g"], "lnng": g["ln_n_g"],
        "w1v": np.ascontiguousarray(w1v).astype(bf16), "b1v": b1v,
        "w2v": np.ascontiguousarray(g["fv_w2"].T).astype(bf16), "b2v": b2v,
        "w1n": np.ascontiguousarray(w1n).astype(bf16), "b1n": b1n,
        "w2n": np.ascontiguousarray(g["fn_w2"].T).astype(bf16), "b2n": b2n,
        "ones_b_d": np.ones((P, P), bf16),
    }
    vT = np.ascontiguousarray(g["verb_features"].T)               # [E, 16384]
    nT = np.ascontiguousarray(g["noun_features"].T)
    vTb = (vT + bat1[:, None]).astype(bf16)
    nTb = (nT + bat2[:, None]).astype(bf16)
    nTq = nT.astype(bf16)
    in_maps = []
    for i in range(NCORES):
        cs = slice(i * B, (i + 1) * B)
        m = dict(common)
        m["vTb"] = np.ascontiguousarray(vTb[:, cs])
        m["nTb"] = np.ascontiguousarray(nTb[:, cs])
        m["nT"] = np.ascontiguousarray(nTq[:, cs])
        in_maps.append(m)
    return in_maps


def kernel(**inputs):
    nc = _get_program()
    in_maps = _prepare_maps(inputs)
    res = run_bass_kernel_spmd(nc, in_maps, list(range(NCORES))).results
    verb = np.concatenate([res[i]["verb_out"] for i in range(NCORES)], axis=1)
    noun = np.concatenate([res[i]["noun_out"] for i in range(NCORES)], axis=1)
    return np.ascontiguousarray(verb.T), np.ascontiguousarray(noun.T)


# revision 48
# speedup vs baseline: 1.1376x; 1.1376x over previous
"""Trainium2 Bass kernel for a dense cross-task transformer block.

Math notes
----------
The reference "attention" has sequence length 1 on the key axis, so
softmax(scores) == 1.0 exactly and the whole q/k/score path is dead:

    mha_len1(q_in, kv_in, ...) == (kv_in @ wv.T + bv) @ wo.T + bo

which folds (on host) into a single matmul with W = wo @ wv and
b = wo @ bv + bo.  The block is then:

    verb1 = LN(verb + noun @ W1.T + c1)          (ln_v)
    verb2 = verb1 + FFN_v(verb1)
    noun1 = LN(noun + verb2 @ W2.T + c2)         (ln_n)
    noun2 = noun1 + FFN_n(noun1)
    return verb2, noun2

Device strategy
---------------
Pure data parallel over 8 cores (batch 16384 -> 2048 cols/core), weights
replicated, everything feature-major ([E, batch]) so matmuls contract
along the SBUF partition dim.

v2 design (vs the f32r baseline):
 - all matmul operands in bf16 (fp32 PSUM accumulation) -> FastWeightLoad
   kicks in and LDWEIGHTS overlaps fully; inputs/weights uploaded bf16.
 - phases stream per 512-column chunk; verb2 stays resident in SBUF for
   phase C (no DRAM round trip).
 - FFN loops are chunk-outer so FFN chunk c starts as soon as the LN of
   chunk c lands; w1 resident per phase, w2 streamed per chunk-halfblock.
 - single 8-bank PSUM pool choreography: mains/ffn1 rotate banks 0-3,
   attn stats use 4-5, LN broadcasts 6-7, ffn2 half-blocks use 0-3/4-7.
 - LayerNorm reduces across partitions via ones-vector matmuls; -mean and
   1/std broadcast back with K=1 matmuls; scale/shift fused in one
   tensor_scalar op.
"""

import os
import numpy as np
import ml_dtypes
from contextlib import ExitStack

import concourse.bass as bass
import concourse.bacc as bacc_mod
import concourse.mybir as mybir
import concourse.tile as tile
from concourse.bass_utils import run_bass_kernel_spmd

E = 1024          # embed dim
H2 = 2048         # FFN hidden dim
B_TOTAL = 16384
NCORES = 8
B = B_TOTAL // NCORES   # 2048 cols per core
P = 128
EPS = 1e-5
CHUNK = 512
NCH = B // CHUNK  # 4
KT = E // P       # 8
MT = E // P       # 8
HT = H2 // P      # 16

F32 = mybir.dt.float32
F32R = mybir.dt.float32r
BF16 = mybir.dt.bfloat16
AF = mybir.ActivationFunctionType
OP = mybir.AluOpType


def _build_program():
    nc = bacc_mod.Bacc("TRN2", target_bir_lowering=False)

    # vTb = verb + attn1_bias, nTb = noun + attn2_bias (bias host-folded
    # into the residual); nT = raw noun features (phase-A matmul input)
    vTb = nc.declare_dram_parameter("vTb", [E, B], BF16, isOutput=False)
    nTb = nc.declare_dram_parameter("nTb", [E, B], BF16, isOutput=False)
    nT = nc.declare_dram_parameter("nT", [E, B], BF16, isOutput=False)
    wat1 = nc.declare_dram_parameter("wat1", [E, E], BF16, isOutput=False)  # (wo@wv).T
    wat2 = nc.declare_dram_parameter("wat2", [E, E], BF16, isOutput=False)
    lnvg = nc.declare_dram_parameter("lnvg", [E], F32, isOutput=False)
    lnng = nc.declare_dram_parameter("lnng", [E], F32, isOutput=False)
    w1v = nc.declare_dram_parameter("w1v", [E, H2], BF16, isOutput=False)   # fv_w1.T
    b1v = nc.declare_dram_parameter("b1v", [H2], F32, isOutput=False)
    w2v = nc.declare_dram_parameter("w2v", [H2, E], BF16, isOutput=False)   # fv_w2.T
    b2v = nc.declare_dram_parameter("b2v", [E], F32, isOutput=False)
    w1n = nc.declare_dram_parameter("w1n", [E, H2], BF16, isOutput=False)
    b1n = nc.declare_dram_parameter("b1n", [H2], F32, isOutput=False)
    w2n = nc.declare_dram_parameter("w2n", [H2, E], BF16, isOutput=False)
    b2n = nc.declare_dram_parameter("b2n", [E], F32, isOutput=False)
    ones_b_d = nc.declare_dram_parameter("ones_b_d", [P, P], BF16, isOutput=False)
    verb_out = nc.declare_dram_parameter("verb_out", [E, B], F32, isOutput=True)
    noun_out = nc.declare_dram_parameter("noun_out", [E, B], F32, isOutput=True)


    with tile.TileContext(nc) as tc, ExitStack() as ctx:
        dramp = ctx.enter_context(tc.tile_pool(name="dramp", bufs=1,
                                               space="DRAM"))
        const = ctx.enter_context(tc.tile_pool(name="const", bufs=1))
        kxp = ctx.enter_context(tc.tile_pool(name="kxp", bufs=3))
        rp = ctx.enter_context(tc.tile_pool(name="rp", bufs=3))
        v1p = ctx.enter_context(tc.tile_pool(name="v1p", bufs=1))
        v2sp = ctx.enter_context(tc.tile_pool(name="v2sp", bufs=4))
        hp = ctx.enter_context(tc.tile_pool(name="hp", bufs=2))
        wap = ctx.enter_context(tc.tile_pool(name="wap", bufs=1))
        w1p = ctx.enter_context(tc.tile_pool(name="w1p", bufs=1))
        w2sp = ctx.enter_context(tc.tile_pool(name="w2sp", bufs=3))
        stp = ctx.enter_context(tc.tile_pool(name="stp", bufs=4))
        sqp = ctx.enter_context(tc.tile_pool(name="sqp", bufs=1))
        smp = ctx.enter_context(tc.tile_pool(name="smp", bufs=1))
        psp = ctx.enter_context(tc.tile_pool(name="psp", bufs=1, space="PSUM"))

        def psum(i):
            return psp.tile([P, CHUNK], F32, tag=f"b{i}", name=f"b{i}")

        def psum_stat(i):
            return psp.tile([1, CHUNK], F32, tag=f"b{i}", name=f"b{i}")

        # full [128,128] ones as the stats lhsT: the column-sum matmul then
        # lands REPLICATED across all 128 partitions -> broadcast for free
        ones_bf = const.tile([P, P], BF16, tag="ones_bf", name="ones_bf")
        nc.sync.dma_start(out=ones_bf[:], in_=ones_b_d[:, :])
        eps_t = const.tile([P, 1], F32, tag="eps", name="eps")
        nc.vector.memset(eps_t[:], EPS)

        def load_pvec(dram_ap, ntiles, tag):
            t = const.tile([P, ntiles], F32, tag=tag, name=tag)
            nc.sync.dma_start(out=t[:], in_=dram_ap.rearrange("(t p) -> p t", p=P))
            return t

        lnvg_pb = load_pvec(lnvg[:], MT, "lnvg")
        lnng_pb = load_pvec(lnng[:], MT, "lnng")
        b1v_pb = load_pvec(b1v[:], HT, "b1v")
        b2v_pb = load_pvec(b2v[:], MT, "b2v")
        b1n_pb = load_pvec(b1n[:], HT, "b1n")
        b2n_pb = load_pvec(b2n[:], MT, "b2n")

        def load_attn_w(w_dram):
            tiles = []
            for k in range(KT):
                t = wap.tile([P, E], BF16, tag=f"aw{k}", name=f"aw{k}")
                nc.sync.dma_start(out=t[:], in_=w_dram[k * P:(k + 1) * P, :])
                tiles.append(t)
            return tiles

        def load_w1(w_dram):
            tiles = []
            for k in range(KT):
                t = w1p.tile([P, H2], BF16, tag=f"w1_{k}", name=f"w1_{k}")
                nc.sync.dma_start(out=t[:], in_=w_dram[k * P:(k + 1) * P, :])
                tiles.append(t)
            return tiles

        def attn_ln(wt, kx_dram, kx_tiles, res_dram, out_tiles):
            """out[m][:,cs] = (x - mean)/std with x = res + W@kx, per
            512-col chunk.  LN gain/shift are host-folded into the
            downstream FFN weights and residual path.

            The stats matmuls use a full ones matrix as lhsT, so the
            column sums arrive replicated across all 128 partitions — the
            whole mean/var/1-over-std chain runs on replicated [128,512]
            tiles and no partition broadcast is ever needed.  Chunk c's
            apply is emitted after chunk c+1's matmuls (software pipeline).
            """
            def ln_apply(c, nmB, rsB):
                cs = slice(c * CHUNK, (c + 1) * CHUNK)
                for m in range(MT):
                    xt = out_tiles[m][:, cs]
                    nc.vector.tensor_add(xt, xt, nmB[:])
                    nc.vector.tensor_mul(xt, xt, rsB[:])

            def varchain(c, stats_x, stats_q):
                """Replicated [128,512] stats -> -mean (bf16), 1/std (f32)."""
                nmB = smp.tile([P, CHUNK], BF16, tag=f"nb{c % 2}",
                               name=f"nb{c % 2}")
                with nc.allow_low_precision(reason="mean in bf16 is plenty"):
                    nc.vector.tensor_scalar(
                        nmB[:], stats_x[:], -1.0 / E, None, OP.mult)
                m2 = smp.tile([P, CHUNK], F32, tag="m2", name="m2")
                nc.scalar.activation(m2[:], stats_x[:], AF.Square, scale=1.0 / E)
                t1 = smp.tile([P, CHUNK], F32, tag="t1", name="t1")
                nc.vector.tensor_scalar(t1[:], stats_q[:], 1.0 / E, None, OP.mult)
                nc.vector.tensor_sub(t1[:], t1[:], m2[:])           # var
                u = smp.tile([P, CHUNK], F32, tag="m2", name="m2")
                nc.scalar.activation(u[:], t1[:], AF.Ln, bias=eps_t[:])
                rsB = smp.tile([P, CHUNK], F32, tag=f"rb{c % 2}",
                               name=f"rb{c % 2}")
                nc.scalar.activation(rsB[:], u[:], AF.Exp, scale=-0.5)
                return nmB, rsB

            # deeper software pipeline across chunks: chunk c's last stats
            # pair + var chain are deferred into chunk c+1's m-loop, and its
            # scale/shift into chunk c+2, so the PE never waits on the
            # DVE/ACT drain chains at chunk boundaries.
            carry = {}   # chunk -> closures/tiles in flight

            def emit_chunk(c):
                cs = slice(c * CHUNK, (c + 1) * CHUNK)
                if kx_tiles is None:
                    kx = []
                    for k in range(KT):
                        t = kxp.tile([P, CHUNK], BF16, tag=f"kx{k}", name=f"kx{k}")
                        nc.sync.dma_start(out=t[:], in_=kx_dram[k * P:(k + 1) * P, cs])
                        kx.append(t[:])
                else:
                    kx = [kx_tiles[k][:, cs] for k in range(KT)]
                rt = []
                for m in range(MT):
                    t = rp.tile([P, CHUNK], BF16, tag=f"r{m}", name=f"r{m}")
                    nc.sync.dma_start(out=t[:], in_=res_dram[m * P:(m + 1) * P, cs])
                    rt.append(t)
                stats_x = psum(4)
                stats_q = psum(5)
                sqs = []

                def stats(m):
                    nc.tensor.matmul(stats_x[:], lhsT=ones_bf[:],
                                     rhs=out_tiles[m][:, cs],
                                     start=(m == 0), stop=(m == MT - 1))
                    nc.tensor.matmul(stats_q[:], lhsT=ones_bf[:],
                                     rhs=sqs[m][:],
                                     start=(m == 0), stop=(m == MT - 1))

                carry[c] = {"stats": stats, "sx": stats_x, "sq": stats_q}
                for m in range(MT):
                    ps = psum((0, 1, 2, 3, 6, 7)[m % 6])
                    for k in range(KT):
                        nc.tensor.matmul(
                            ps[:], lhsT=wt[k][:, m * P:(m + 1) * P],
                            rhs=kx[k],
                            start=(k == 0), stop=(k == KT - 1))
                    xt = out_tiles[m][:, cs]
                    nc.vector.tensor_add(xt, ps[:], rt[m][:])
                    sqm = sqp.tile([P, CHUNK], BF16, tag=f"sq{m % 2}",
                                   name=f"sq{m % 2}")
                    nc.scalar.activation(sqm[:], xt, AF.Square)
                    sqs.append(sqm)
                    if m > 0:
                        stats(m - 1)
                    if m == 0 and c > 0:
                        pc = carry[c - 1]
                        pc["stats"](MT - 1)
                        pc["bc"] = varchain(c - 1, pc["sx"], pc["sq"])
                    if m == 1 and c > 1:
                        ln_apply(c - 2, *carry.pop(c - 2)["bc"])

            for c in range(NCH):
                emit_chunk(c)
            last = carry[NCH - 1]
            last["stats"](MT - 1)
            last["bc"] = varchain(NCH - 1, last["sx"], last["sq"])
            if NCH > 1:
                ln_apply(NCH - 2, *carry.pop(NCH - 2)["bc"])
            ln_apply(NCH - 1, *carry.pop(NCH - 1)["bc"])

        def ffn(in_tiles, w1t, b1_pb, w2_dram, g_pb, b2_pb, out_dram, out_bf):
            """out = (in*g + b2') + W2.T@gelu(W1'.T@in + b1') per 512-col
            chunk.  in_tiles hold the unscaled LN output; the LN gain is
            host-folded into W1 and applied to the residual here (g_pb),
            with b2' = ln_bias + ffn_b2."""
            for c in range(NCH):
                cs = slice(c * CHUNK, (c + 1) * CHUNK)
                hts = []
                for hm in range(HT):
                    ps = psum(hm % 4)
                    for k in range(KT):
                        nc.tensor.matmul(
                            ps[:], lhsT=w1t[k][:, hm * P:(hm + 1) * P],
                            rhs=in_tiles[k][:, cs],
                            start=(k == 0), stop=(k == KT - 1))
                    ht = hp.tile([P, CHUNK], BF16, tag=f"h{hm}", name=f"h{hm}")
                    nc.scalar.activation(ht[:], ps[:], AF.Gelu,
                                         bias=b1_pb[:, hm:hm + 1])
                    hts.append(ht)
                pss = [psum(m) for m in range(MT)]
                for k in range(HT):
                    w2t = w2sp.tile([P, E], BF16, tag="w2s", name="w2s")
                    nc.sync.dma_start(out=w2t[:],
                                      in_=w2_dram[k * P:(k + 1) * P, :])
                    for m in range(MT):
                        nc.tensor.matmul(
                            pss[m][:], lhsT=w2t[:, m * P:(m + 1) * P],
                            rhs=hts[k][:],
                            start=(k == 0), stop=(k == HT - 1))
                for m in range(MT):
                    st = stp.tile([P, CHUNK], F32, tag="st", name="st")
                    nc.vector.tensor_scalar(
                        st[:], in_tiles[m][:, cs], g_pb[:, m:m + 1],
                        b2_pb[:, m:m + 1], OP.mult, OP.add)
                    nc.vector.tensor_add(st[:], st[:], pss[m][:])
                    nc.sync.dma_start(out=out_dram[m * P:(m + 1) * P, cs],
                                      in_=st[:])
                    if out_bf is not None:
                        vb = v2sp.tile([P, CHUNK], BF16, tag="v2s",
                                       name="v2s")
                        nc.scalar.activation(vb[:], st[:], AF.Copy)
                        nc.sync.dma_start(
                            out=out_bf[m * P:(m + 1) * P, cs], in_=vb[:])

        _REP = int(os.environ.get("BENCH_REPEAT", "1"))
        v2d_t = dramp.tile([E, B], BF16, tag="v2d", name="v2d")
        v2d = v2d_t[:, :]
        for _rep in range(_REP):
            # A: verb attends to noun, LN -> verb1 (SBUF resident)
            wA = load_attn_w(wat1)
            v1 = [v1p.tile([P, B], BF16, tag=f"v1_{m}", name=f"v1_{m}")
                  for m in range(MT)]
            attn_ln(wA, nT, None, vTb, v1)
            w1tv = load_w1(w1v)
            # B: verb FFN -> verb_out (DRAM, f32) + v2d (DRAM bf16)
            ffn(v1, w1tv, b1v_pb, w2v, lnvg_pb, b2v_pb, verb_out, v2d)
            # C: noun attends to verb2 (read back), LN -> noun1 (v1 slots)
            wC = load_attn_w(wat2)
            w1tn = load_w1(w1n)
            n1 = [v1p.tile([P, B], BF16, tag=f"v1_{m}", name=f"v1_{m}")
                  for m in range(MT)]
            attn_ln(wC, v2d, None, nTb, n1)
            # D: noun FFN -> noun_out
            ffn(n1, w1tn, b1n_pb, w2n, lnng_pb, b2n_pb, noun_out, None)

    nc.finalize()
    return nc


_prog_cache = {}


def _get_program():
    if "nc" not in _prog_cache:
        _prog_cache["nc"] = _build_program()
    return _prog_cache["nc"]


def _prepare_maps(inputs):
    f32 = np.float32
    bf16 = ml_dtypes.bfloat16
    g = {k: np.asarray(v, f32) for k, v in inputs.items()}

    def fold(p):
        w = g[f"{p}_wo"] @ g[f"{p}_wv"]
        b = g[f"{p}_wo"] @ g[f"{p}_bv"] + g[f"{p}_bo"]
        return np.ascontiguousarray(w.T).astype(bf16), np.ascontiguousarray(b)

    wat1, bat1 = fold("v2n")
    wat2, bat2 = fold("n2v")
    # LN gain/shift fold: FFN runs on the unscaled normalized x, with
    #   W1' = W1 * g (per input feature), b1' = b1 + W1 @ beta,
    #   residual applied as in*g + (beta + b2) on device.
    w1v = (g["fv_w1"] * g["ln_v_g"][None, :]).T
    b1v = g["fv_b1"] + g["fv_w1"] @ g["ln_v_b"]
    b2v = g["fv_b2"] + g["ln_v_b"]
    w1n = (g["fn_w1"] * g["ln_n_g"][None, :]).T
    b1n = g["fn_b1"] + g["fn_w1"] @ g["ln_n_b"]
    b2n = g["fn_b2"] + g["ln_n_b"]
    common = {
        "wat1": wat1, "wat2": wat2,
        "lnvg": g["ln_v_g"], "lnng": g["ln_n_g"],
        "w1v": np.ascontiguousarray(w1v).astype(bf16), "b1v": b1v,
        "w2v": np.ascontiguousarray(g["fv_w2"].T).astype(bf16), "b2v": b2v,
        "w1n": np.ascontiguousarray(w1n).astype(bf16), "b1n": b1n,
        "w2n": np.ascontiguousarray(g["fn_w2"].T).astype(bf16), "b2n": b2n,
        "ones_b_d": np.ones((P, P), bf16),
    }
    vT = np.ascontiguousarray(g["verb_features"].T)               # [E, 16384]
    nT = np.ascontiguousarray(g["noun_features"].T)
    vTb = (vT + bat1[:, None]).astype(bf16)
    nTb = (nT + bat2[:, None]).astype(bf16)
    nTq = nT.astype(bf16)
    in_maps = []
    for i in range(NCORES):
        cs = slice(i * B, (i + 1) * B)
        m = dict(common)
        m["vTb"] = np.ascontiguousarray(vTb[:, cs])
        m["nTb"] = np.ascontiguousarray(nTb[:, cs])
        m["nT"] = np.ascontiguousarray(nTq[:, cs])
        in_maps.append(m)
    return in_maps


def kernel(**inputs):
    nc = _get_program()
    in_maps = _prepare_maps(inputs)
    res = run_bass_kernel_spmd(nc, in_maps, list(range(NCORES))).results
    verb = np.concatenate([res[i]["verb_out"] for i in range(NCORES)], axis=1)
    noun = np.concatenate([res[i]["noun_out"] for i in range(NCORES)], axis=1)
    return np.ascontiguousarray(verb.T), np.ascontiguousarray(noun.T)
